# revision 1
# baseline (speedup 1.0000x reference)
"""DTW loss kernel for Trainium2 (Bass), 8-core data-parallel.

Problem: mean over batch B=64 of DTW path cost with L1 point distance,
sequences pred/target of shape [64, 512, 2] fp32.

Sharding: pure data parallel — each of the 8 cores runs the DTW DP for its
8 sequences; the scalar mean is reduced on host from the 64 terminal values.

Per-core algorithm (anti-diagonal wavefront over column blocks):
  DP: D[i,j] = C[i,j] + min(D[i-1,j], D[i-1,j-1], D[i,j-1]),
      C[i,j] = |p0[i]-t0[j]| + |p1[i]-t1[j]|.
  The row is split into K=16 blocks of W=32 columns. SBUF lane p = b*16 + k
  (b: local sequence, k: column block). At wavefront step t (0..526) lane
  (b,k) computes DP row i = t-k of its block with one hardware scan:
    stream_shuffle: carry candidate from lane p-1 (left block's last column)
    tensor_tensor(min): upmin[j] = min(D[i-1, j-1], D[i-1, j])
    tensor_tensor_scan(min, add): state = min(upmin[j], state) + C[i,j]
  The scan runs over W+1 elements; element 0 regenerates the carry
  D[i, k*W-1] as min(shuffled, BIG) + maskadd (maskadd=BIG on k=0 lanes
  forces the row-left boundary to +inf).

  C rows are bulk-produced in chunks of CH=64 wavefront steps with
  free-dim-broadcast APs: Pool computes t - p differences for a whole chunk
  in one tensor_tensor, ACT applies Abs, Pool accumulates the two
  components into a double-buffered chunk tile whose 33-wide slices are
  [maskadd | C row]. A pre-shifted pred layout (ps[p, t] = pred[b, t-k],
  padded with BIG outside the valid range) gives every lane its row scalar
  at free offset t. One DVE tensor_copy per chunk observes Pool's final
  write so the scans never need a cross-engine semaphore (the TensorScalar
  and CTRL ISA encodings have a single sync-wait slot).

  All per-core inputs are packed into one blob (single DMA, loaded before
  the TileContext with a manual semaphore handshake).
"""

import numpy as np

B, N, ND = 64, 512, 2
NCORES = 8
BPC = B // NCORES            # 8 sequences per core
K = 16                       # column blocks per row
W = N // K                   # 32 columns per block
SW = W + 1                   # chunk slice width: [maskadd | C row]
P = BPC * K                  # 128 lanes
T = N + K - 1                # 527 wavefront steps
BIG = 1.0e30
CH = 32                      # wavefront steps per C chunk
SHIFT_MASK = [(i - 1) % 32 for i in range(32)]

# blob column layout
_PS0, _PS1 = 0, T
_T0, _T1 = 2 * T, 2 * T + W
_MASK = 2 * T + 2 * W
_BINITB = _MASK + 1
BLOB_F = _BINITB + SW

_CACHE: dict = {}


def _build_program():
    import contextlib

    import concourse.bass as bass
    import concourse.mybir as mybir
    from concourse.tile import TileContext
    from concourse.tile_rust import add_dep_helper

    f32 = mybir.dt.float32
    nc = bass.Bass("TRN2", debug=False, enable_asserts=False)

    blob_d = nc.dram_tensor("blob", [P, BLOB_F], f32, kind="ExternalInput").ap()
    out_d = nc.dram_tensor("out_d", [P, 1], f32, kind="ExternalOutput").ap()
    outsb = nc.alloc_sbuf_tensor("outsb", [P, 1], f32).ap()
    blob = nc.alloc_sbuf_tensor("blobsb", [P, BLOB_F], f32).ap()

    mn, ad, sub = mybir.AluOpType.min, mybir.AluOpType.add, mybir.AluOpType.subtract
    AF = mybir.ActivationFunctionType

    ps0 = blob[:, _PS0 : _PS0 + T]
    ps1 = blob[:, _PS1 : _PS1 + T]
    t0 = blob[:, _T0 : _T0 + W]
    t1 = blob[:, _T1 : _T1 + W]

    # Load the input blob before the TileContext with a manual semaphore
    # handshake: keeps the DMA proc out of Tile's tail drain (CTRL sync-wait
    # slots are scarce).
    _stack = contextlib.ExitStack()
    sem = _stack.enter_context(nc.semaphore())
    nc.sync.dma_start(blob, blob_d[:]).then_inc(sem, 16)
    nc.gpsimd.wait_ge(sem, 16)
    nc.vector.wait_ge(sem, 16)
    nc.scalar.wait_ge(sem, 16)

    # chunk-size ramp: small leading chunks let the DVE wavefront start
    # ~3us in instead of waiting ~22us for a full 64-step C chunk
    chs_list = [8, 8, 16]
    rem = T - sum(chs_list)
    while rem > 0:
        c = min(CH, rem)
        chs_list.append(c)
        rem -= c

    with TileContext(nc) as tc:
        with tc.tile_pool(name="pers", bufs=1) as pool:
            bufA = pool.tile([P, SW], f32, tag="bufA")
            bufB = pool.tile([P, SW], f32, tag="bufB")
            umbuf = pool.tile([P, SW], f32, tag="umbuf")
            csync = pool.tile([P, 1], f32, tag="csync")
            cbuf = [
                pool.tile([P, CH * SW], f32, name=f"cbuf{i}", tag=f"cbuf{i}")
                for i in range(2)
            ]
            d0scr = [
                pool.tile([P, CH * W], f32, name=f"d0s{i}", tag=f"d0s{i}")
                for i in range(2)
            ]
            d1scr = [
                pool.tile([P, CH * W], f32, name=f"d1s{i}", tag=f"d1s{i}")
                for i in range(2)
            ]
            a1scr = [
                pool.tile([P, CH * W], f32, name=f"a1s{i}", tag=f"a1s{i}")
                for i in range(2)
            ]

            # initial D row image: col0 = 0 on k=0 lanes else BIG, rest BIG
            nc.gpsimd.tensor_copy(bufB[:], blob[:, _BINITB : _BINITB + SW])
            # maskadd into col 0 of every 33-wide slice of both chunk bufs
            for i in range(2):
                dst = cbuf[i][:].rearrange("p (s j) -> p s j", j=SW)[:, :, 0:1]
                src = blob[:, _MASK : _MASK + 1].unsqueeze(1).broadcast_to(
                    [P, CH, 1]
                )
                nc.gpsimd.tensor_copy(dst, src)

            tg = 0
            for g, ch in enumerate(chs_list):
                cb = cbuf[g % 2]
                d0, d1, a1 = d0scr[g % 2], d1scr[g % 2], a1scr[g % 2]
                c_rows = cb[:].rearrange("p (s j) -> p s j", j=SW)[
                    :, 0:ch, 1 : W + 1
                ]
                # Pool: per-chunk differences via free-dim broadcast
                t0b = t0.unsqueeze(1).broadcast_to([P, ch, W])
                t1b = t1.unsqueeze(1).broadcast_to([P, ch, W])
                p0b = ps0[:, tg : tg + ch].unsqueeze(2).broadcast_to([P, ch, W])
                p1b = ps1[:, tg : tg + ch].unsqueeze(2).broadcast_to([P, ch, W])
                d0v = d0[:, 0 : ch * W].rearrange("p (s j) -> p s j", j=W)
                a1v = a1[:, 0 : ch * W].rearrange("p (s j) -> p s j", j=W)
                # comp0: ACT per step (no SBUF-port contention with DVE);
                # comp1: DVE bulk sub + one bulk ACT Abs per chunk, keeping
                # ACT below the DVE step rate. One bulk DVE add folds them
                # (same-engine -> scans need no sem).
                d1v = d1[:, 0 : ch * W].rearrange("p (s j) -> p s j", j=W)
                nc.vector.tensor_tensor(d1v, t1b, p1b, op=sub)
                nc.scalar.activation(a1v, d1v, AF.Abs)
                for s in range(ch):
                    t = tg + s
                    nc.scalar.activation(
                        d0[:, s * W : (s + 1) * W], t0, AF.Abs,
                        bias=ps0[:, t : t + 1], scale=1.0,
                    )
                nc.vector.tensor_tensor(c_rows, d0v, a1v, op=ad)

                for s in range(ch):
                    t = tg + s
                    bcur, bprev = (bufA, bufB) if t % 2 == 0 else (bufB, bufA)
                    sh = nc.vector.stream_shuffle(
                        umbuf[:, 0:1], bprev[:, W : W + 1], SHIFT_MASK
                    )
                    nc.vector.tensor_tensor(
                        umbuf[:, 1:SW], bprev[:, 0:W], bprev[:, 1:SW], op=mn
                    )
                    nc.vector.tensor_tensor_scan(
                        bcur[:, 0:SW], umbuf[:, 0:SW],
                        cb[:, s * SW : (s + 1) * SW],
                        float(BIG), op0=mn, op1=ad,
                    )

                tg += ch

            final = bufA if (T - 1) % 2 == 0 else bufB
            nc.vector.tensor_copy(outsb, final[:, W : W + 1])

    # Past the TileContext tail barrier every engine is quiesced, so the raw
    # SP-issued output DMA needs no data-dependency semaphores; its own
    # completion semaphore (required by DGE codegen) doubles as the final
    # flush before the NEFF completes.
    nc.sync.dma_start(out_d[:], outsb).then_inc(sem, 32)
    nc.sync.wait_ge(sem, 48)
    _stack.close()
    _split_multi_waits(nc, mybir)
    return nc


def _split_multi_waits(nc, mybir, cap=1):
    """Walrus CTRL/TensorScalar encodings accept a single sync-wait; Tile
    occasionally emits more on its tail drain. Hoist extras onto same-engine
    no-ops placed immediately before the offending instruction."""
    fn = nc.m.functions[0]
    for blk in fn.blocks:
        insts = list(blk.instructions)
        new = []
        changed = False
        for inst in insts:
            si = getattr(inst, "sync_info", None)
            waits = list(si.on_wait) if si and si.on_wait else []
            if len(waits) > cap:
                for i, w in enumerate(waits[:-cap]):
                    new.append(
                        mybir.InstNoOp(
                            name=f"{inst.name}-wsplit{i}",
                            sync_info=mybir.SyncInfo(on_wait=[w], on_update=[]),
                            engine=inst.engine,
                            bass_nofuse=True,
                        )
                    )
                si.on_wait = waits[-cap:]
                changed = True
            new.append(inst)
        if changed:
            blk.instructions = new


def _host_prep(pred_c: np.ndarray, target_c: np.ndarray) -> dict:
    """pred_c, target_c: [BPC, N, 2] float32 -> one core's input blob."""
    blob = np.full((P, BLOB_F), BIG, np.float32)
    # ps regions hold NEGATED pred (ACT computes Abs(t + bias), bias = -p);
    # pad with -BIG so padded cells become ~BIG after Abs
    blob[:, _PS0 : _PS0 + T] = -BIG
    for k in range(K):
        blob[k::K, _PS0 + k : _PS0 + k + N] = -pred_c[:, :, 0]
    for k in range(K):
        blob[k::K, _PS1 + k : _PS1 + k + N] = pred_c[:, :, 1]
    tt = target_c.reshape(BPC, K, W, ND)
    blob[:, _T0 : _T0 + W] = tt[:, :, :, 0].reshape(P, W)
    blob[:, _T1 : _T1 + W] = tt[:, :, :, 1].reshape(P, W)
    lane_k0 = (np.arange(P) % K) == 0
    blob[:, _MASK] = np.where(lane_k0, BIG, 0.0)
    blob[:, _BINITB:] = BIG
    blob[:, _BINITB] = np.where(lane_k0, 0.0, BIG)
    return {"blob": blob}


def _run(in_maps, trace=False):
    from concourse.bass_utils import run_bass_kernel_spmd

    if "nc" not in _CACHE:
        _CACHE["nc"] = _build_program()
    return run_bass_kernel_spmd(
        _CACHE["nc"], in_maps, core_ids=list(range(NCORES)), trace=trace
    )


def kernel(pred: np.ndarray, target: np.ndarray, _trace=False):
    pred = np.asarray(pred, np.float32)
    target = np.asarray(target, np.float32)
    in_maps = [
        _host_prep(pred[c * BPC : (c + 1) * BPC], target[c * BPC : (c + 1) * BPC])
        for c in range(NCORES)
    ]
    res = _run(in_maps, trace=_trace)
    vals = np.concatenate(
        [r["out_d"][K - 1 :: K, 0] for r in res.results]
    ).astype(np.float64)
    out = np.float32(vals.mean())
    if _trace:
        return out, res
    return out

